# revision 12
# baseline (speedup 1.0000x reference)
"""Trainium2 Bass kernel for nn_Decoder_71554155151955 (SwitchedConv decoder).

Data-parallel over batch: 8 batch elements -> 8 NeuronCores, identical program.
Per core: full decoder forward in SBUF, position-major matmuls in float32r.

Design notes:
 - 64-res feature maps are kept channel-major as THREE horizontally shifted
   copies S[dx], dx in {-1, 0, +1}, each [128, 66*64] with zero rows 0 and 65
   (vertical pad) and a zero column folded in by the shift DMAs.  A 3x3 tap
   (dy, dx) for an output block of two rows starting at r0 is then the
   contiguous slice S[dx-1][:, (r0+dy)*64 : +128]  (M=128, single stride).
   S[-1]/S[+1] are rebuilt from S[0] by two SBUF->SBUF DMAs per layer.
 - The up1 output (128-res) is channel-major with a width-129 zero-spacer
   layout: addr(b, c) = 1 + b*129 + c; taps for one 128-wide output row are
   single-stride slices.
 - Main matmuls are position-major: stationary = input positions [K=ci, M=128],
   moving = weights [K=ci, N=(v,co)<=512], taps accumulate in PSUM; a K=1
   matmul against a ones-row seeds the accumulator with the bias.
 - Per-position softmax gates live in [pos, breadth] layout; the breadth-gated
   sum is 8 fused DVE scalar_tensor_tensor ops; PE transposes the gated
   [pos, co] block back to channel-major for the next layer.
 - float32r everywhere on the matmul path (full PE rate, ~11-bit mantissa);
   fp32r PSUM destinations must start at partition 0 (hardware restriction),
   which the M=128 design guarantees.
"""
import numpy as np
from contextlib import ExitStack

B, CIN, H0, W0 = 8, 128, 64, 64
CH, NRC, BR, COUT = 128, 64, 8, 3

SZS = 66 * 64 + 72       # shifted-image buffer: 66 rows x 64 cols + slack
PW2, NB2 = 129, 130      # 128-res spacer layout
SZ128 = 1 + NB2 * PW2 + 260

_cache = {}


def _phase_kernels(W5):
    """[co, ci, 5, 5] -> {(di, dj): [co, ci, 3, 3]} for nearest-up2x + conv."""
    grp = {0: [(0, 0), (1, 0), (2, 1), (3, 1), (4, 2)],
           1: [(0, 0), (1, 1), (2, 1), (3, 2), (4, 2)]}
    out = {}
    for di in (0, 1):
        for dj in (0, 1):
            Wd = np.zeros(W5.shape[:2] + (3, 3), np.float32)
            for ty, s in grp[di]:
                for tx, t in grp[dj]:
                    Wd[:, :, s, t] += W5[:, :, ty, tx]
            out[(di, dj)] = Wd
    return out


def _prep_weights(inputs):
    d = {}

    def conv_layer(name, params, k):
        Wt, b, cw, cb = [np.ascontiguousarray(np.asarray(p), np.float32) for p in params]
        br, co, ci, _, _ = Wt.shape
        W = Wt.transpose(3, 4, 2, 0, 1).reshape(k * k, ci, br * co)
        d[f"w_{name}"] = np.ascontiguousarray(W.transpose(1, 0, 2).reshape(ci, -1))
        d[f"b_{name}"] = b.reshape(1, br * co).copy()
        d[f"cw_{name}"] = np.ascontiguousarray(cw[:, :, 0, 0].T)
        d[f"cb_{name}"] = np.tile(cb.astype(np.float32), (128, 32)).copy()

    conv_layer("conv0", inputs["conv0"], 3)
    conv_layer("c1a", inputs["rb0_c1"], 3)
    conv_layer("c2a", inputs["rb0_c2"], 1)
    conv_layer("c1b", inputs["rb1_c1"], 3)
    conv_layer("c2b", inputs["rb1_c2"], 1)

    Wt, b, cw, cb = [np.asarray(p).astype(np.float32) for p in inputs["up1"]]
    pk = _phase_kernels(Wt.reshape(BR * 64, CH, 5, 5))
    arr = np.stack([pk[(0, 0)], pk[(0, 1)], pk[(1, 0)], pk[(1, 1)]], 0)
    arr = arr.transpose(3, 4, 2, 0, 1).reshape(9, CH, 4 * 512)  # [(s,t), ci, (ph,v,co)]
    d["w_up1"] = np.ascontiguousarray(arr.transpose(1, 0, 2).reshape(CH, -1))
    d["b_up1"] = np.tile(b.reshape(1, 512), (1, 4)).copy()
    d["cw_up1"] = np.ascontiguousarray(cw[:, :, 0, 0].T)
    d["cb_up1"] = np.tile(cb.astype(np.float32), (128, 32)).copy()

    Wt, b, cw, cb = [np.asarray(p).astype(np.float32) for p in inputs["up2"]]
    pk = _phase_kernels(Wt.reshape(BR * COUT, NRC, 5, 5))
    Bst = np.stack([pk[(0, 0)], pk[(0, 1)], pk[(1, 0)], pk[(1, 1)]], 0)
    Bst = Bst.reshape(2, 2, BR, COUT, NRC, 3, 3)       # (di, dj, v, c, ci, s, t)
    A = Bst.transpose(5, 6, 4, 2, 1, 3, 0)             # (s, t, ci, v, dj, c, di)
    A = np.ascontiguousarray(A).reshape(9, NRC, 96)
    d["w_up2"] = np.ascontiguousarray(A.transpose(1, 0, 2).reshape(NRC, -1))
    bb = np.zeros((BR, 2, COUT, 2), np.float32)
    bb[:, :, :, :] = b[:, None, :, None]               # (v, dj, c, di)
    d["b_up2"] = bb.reshape(1, 96).copy()
    d["cw_up2"] = np.ascontiguousarray(cw[:, :, 0, 0].T)
    d["cb_up2"] = np.tile(cb.astype(np.float32), (128, 32)).copy()

    d["ones"] = np.ones((1, 128), np.float32)
    d["zeros"] = np.zeros((128, SZ128), np.float32)
    return d


def _win2(ap, off, step_r, nr, step_c, ncnt):
    """2D strided free view [P, nr, ncnt]: addr = off + r*step_r + c*step_c."""
    inner = step_r // step_c
    width = step_r * nr
    v = ap[:, off:off + width].rearrange("p (r c t) -> p r c t",
                                        r=nr, c=inner, t=step_c)
    return v[:, :, 0:ncnt, 0:1].squeeze(3)


def _build(debug=(), layers="all"):
    import concourse.bass as bass
    from concourse import bacc, mybir
    import concourse.tile as tile
    from concourse.masks import make_identity

    f32 = mybir.dt.float32
    f32r = mybir.dt.float32r
    AF = mybir.ActivationFunctionType
    ALU = mybir.AluOpType
    AX = mybir.AxisListType

    nc = bacc.Bacc("TRN2", target_bir_lowering=False, debug=False)

    def din(name, shape, dt=f32r):
        return nc.dram_tensor(name, list(shape), dt, kind="ExternalInput").ap()

    x_d = din("x", (128, 4096))
    NVCO = {"conv0": 1024, "c1a": 512, "c2a": 1024, "c1b": 512, "c2b": 1024,
            "up1": 2048, "up2": 96}
    w = {}
    for name, ci, ntap in [("conv0", 128, 9), ("c1a", 128, 9), ("c2a", 64, 1),
                           ("c1b", 128, 9), ("c2b", 64, 1), ("up1", 128, 9),
                           ("up2", 64, 9)]:
        w[f"w_{name}"] = din(f"w_{name}", (ci, ntap * NVCO[name]))
        w[f"b_{name}"] = din(f"b_{name}", (1, NVCO[name]))
        w[f"cw_{name}"] = din(f"cw_{name}", (ci, 8))
        w[f"cb_{name}"] = din(f"cb_{name}", (128, 256), f32)
    ones_d = din("ones", (1, 128))
    zeros_d = din("zeros", (128, SZ128))
    out_d = nc.dram_tensor("out", [COUT, 256, 256], f32, kind="ExternalOutput").ap()
    dbg_d = {nm: nc.dram_tensor(f"dbg_{nm}", [p, n], f32, kind="ExternalOutput").ap()
             for nm, p, n in debug}

    with tile.TileContext(nc) as tc, ExitStack() as top:
        pers = top.enter_context(tc.tile_pool(name="pers", bufs=1))
        sr = [pers.tile([128, SZS], f32r, tag=f"sr{i}", name=f"sr{i}") for i in range(3)]
        ident = pers.tile([128, 128], f32, tag="ident")
        ones_s = pers.tile([1, 128], f32r, tag="ones")
        gpool = top.enter_context(tc.tile_pool(name="gates", bufs=1))

        zs = pers.tile([128, 144], f32, tag="zs")
        make_identity(nc, ident[:])
        nc.vector.memset(zs[:], 0.0)
        nc.sync.dma_start(ones_s[:], ones_d[:])

        def _col66(ap, off):
            # [128, 66] view of one column (stride 64) at offset `off`
            v = ap[:, off:off + 64 * 66].rearrange("p (r c) -> p r c", r=66, c=64)
            return v[:, :, 0:1].squeeze(2)

        def init_sbuf_zeros(s):
            """zero rows 0 and 65 + tail slack of an S set (engine writes, so
            downstream DMA/matmul wait lists stay short)."""
            sm1, s0, sp1 = s
            for t in (s0, sm1, sp1):
                nc.gpsimd.tensor_copy(t[:, 0:64], zs[:, 0:64])
                nc.gpsimd.tensor_copy(t[:, 65 * 64:SZS], zs[:, 0:64 + 72])

        def rebuild_shifts(s):
            """S[-1], S[+1] from S[0] by whole-buffer shifted copies on the (idle)
            GPSIMD engine; the row-wrap garbage lands exactly on the shift
            zero-columns, which are then re-zeroed by strided writes."""
            sm1, s0, sp1 = s
            nc.gpsimd.tensor_copy(sp1[:, 0:SZS - 1], s0[:, 1:SZS])
            nc.gpsimd.tensor_copy(_col66(sp1[:], 63), zs[:, 0:66])
            nc.gpsimd.tensor_copy(sm1[:, 1:SZS], s0[:, 0:SZS - 1])
            nc.gpsimd.tensor_copy(_col66(sm1[:], 0), zs[:, 0:66])

        def load_w(pool, name, ci, ntap):
            t = {}
            nv = NVCO[name]
            t["w"] = pool.tile([ci, ntap * nv], f32r, tag=f"w_{name}", name=f"w_{name}")
            nc.sync.dma_start(t["w"][:], w[f"w_{name}"][:])
            t["b"] = pool.tile([1, nv], f32r, tag=f"b_{name}", name=f"b_{name}")
            nc.sync.dma_start(t["b"][:], w[f"b_{name}"][:])
            t["cw"] = pool.tile([ci, 8], f32r, tag=f"cw_{name}", name=f"cw_{name}")
            nc.sync.dma_start(t["cw"][:], w[f"cw_{name}"][:])
            t["cb"] = pool.tile([128, 256], f32, tag=f"cb_{name}", name=f"cb_{name}")
            nc.sync.dma_start(t["cb"][:], w[f"cb_{name}"][:])
            return t

        TAPS = [(dy, dx) for dy in range(3) for dx in range(3)]

        def softmax_gates(psg, cb, nblk, gtag="g64"):
            """softmax over groups of 8 of (psg + cb) -> SBUF [128, 8*nblk]."""
            g = gpool.tile([128, 8 * nblk], f32, tag=gtag, name=gtag)
            nc.vector.tensor_tensor(g[:], psg, cb[:, :8 * nblk], op=ALU.add)
            e = gpool.tile([128, 8 * nblk], f32, tag="e_sm", name="e_sm")
            nc.scalar.activation(e[:], g[:], AF.Exp)
            ssum = gpool.tile([128, nblk], f32, tag="ssum", name="ssum")
            nc.vector.tensor_reduce(ssum[:], e[:].rearrange("p (g v) -> p g v", v=8),
                                    AX.X, ALU.add)
            rec = gpool.tile([128, nblk], f32, tag="rec", name="rec")
            nc.vector.reciprocal(rec[:], ssum[:])
            for blk in range(nblk):
                nc.vector.tensor_scalar_mul(g[:, 8 * blk:8 * blk + 8],
                                            e[:, 8 * blk:8 * blk + 8],
                                            rec[:, blk:blk + 1])
            return g

        def gates_64(src_center, lt, psgp):
            """src_center(blk) -> lhsT AP [ci, 128] for the 1x1 coupler."""
            psg = psgp.tile([128, 256], f32, tag="psg", name="psg")
            for blk in range(32):
                nc.tensor.matmul(psg[:, 8 * blk:8 * blk + 8], src_center(blk),
                                 lt["cw"][:], start=True, stop=True)
            return softmax_gates(psg[:], lt["cb"], 32)

        def gated_acc(accp, slices, g, blk, co):
            acc = accp.tile([128, co], f32, tag="acc", name="acc")
            nc.vector.tensor_scalar_mul(acc[:], slices(0), g[:, 8 * blk:8 * blk + 1])
            for v in range(1, 8):
                nc.vector.scalar_tensor_tensor(acc[:], slices(v),
                                               g[:, 8 * blk + v:8 * blk + v + 1],
                                               acc[:], ALU.mult, ALU.add)
            return acc

        def sconv64(lhsT_fn, center_fn, lt, nvco, ntap, dst_write, name):
            """64-res sconv body.  lhsT_fn(blk, t) -> stationary AP [ci, 128]."""
            with ExitStack() as ly:
                psgp = ly.enter_context(tc.tile_pool(name=f"psg_{name}", bufs=1, space="PSUM"))
                g = gates_64(center_fn, lt, psgp)
                psy = ly.enter_context(tc.tile_pool(name=f"psy_{name}", bufs=2, space="PSUM"))
                pst = ly.enter_context(tc.tile_pool(name=f"pst_{name}", bufs=2, space="PSUM"))
                accp = ly.enter_context(tc.tile_pool(name=f"acc_{name}", bufs=3))
                nchunks = (nvco + 511) // 512
                ncs = nvco // nchunks
                co = nvco // 8
                for blk in range(32):
                    ys = [psy.tile([128, ncs], f32, tag=f"y{c}", name=f"y{c}")
                          for c in range(nchunks)]
                    for cnk in range(nchunks):
                        nc.tensor.matmul(ys[cnk][:], ones_s[:],
                                         lt["b"][:, ncs * cnk:ncs * (cnk + 1)],
                                         start=True, stop=False)
                    for t in range(ntap):
                        lhsT = lhsT_fn(blk, t)
                        for cnk in range(nchunks):
                            nc.tensor.matmul(
                                ys[cnk][:], lhsT,
                                lt["w"][:, (t * nchunks + cnk) * ncs:(t * nchunks + cnk + 1) * ncs],
                                start=False, stop=t == ntap - 1)

                    def yslice(v, ys=ys, co=co, ncs=ncs):
                        lo = v * co
                        cnk = lo // ncs
                        return ys[cnk][:, lo - cnk * ncs:lo - cnk * ncs + co]

                    acc = gated_acc(accp, yslice, g, blk, co)
                    pt = pst.tile([co, 128], f32, tag="pt", name="pt")
                    nc.tensor.transpose(pt[:], acc[:], ident[:])
                    dst_write(blk, pt)

        def s_tap(s, blk, t):
            dy, dx = TAPS[t]
            return s[dx][:, (2 * blk + dy) * 64:(2 * blk + dy) * 64 + 128]

        def s_center(s, blk):
            return s[1][:, (2 * blk + 1) * 64:(2 * blk + 1) * 64 + 128]

        init_sbuf_zeros(sr)

        with ExitStack() as hscope:
            hp = hscope.enter_context(tc.tile_pool(name="hp", bufs=1))
            h_buf = hp.tile([128, 4096], f32, tag="h_buf")
            r1_buf = hp.tile([64, 4096], f32r, tag="r1_buf")

            # -------------------- conv0
            with ExitStack() as s0sc:
                xw = s0sc.enter_context(tc.tile_pool(name="xw", bufs=1))
                sx = [xw.tile([128, SZS], f32r, tag=f"sx{i}", name=f"sx{i}") for i in range(3)]
                init_sbuf_zeros(sx)
                nc.sync.dma_start(sx[1][:, 64:64 + 4096], x_d[:])
                rebuild_shifts(sx)
                lt0 = load_w(s0sc.enter_context(tc.tile_pool(name="w0", bufs=1)), "conv0", 128, 9)

                def wb_conv0(blk, pt):
                    m0 = 128 * blk
                    nc.scalar.copy(h_buf[:, m0:m0 + 128], pt[:])
                    nc.scalar.activation(sr[1][:, 64 + m0:64 + m0 + 128], pt[:], AF.Relu)

                sconv64(lambda blk, t: s_tap(sx, blk, t), lambda blk: s_center(sx, blk),
                        lt0, 1024, 9, wb_conv0, "conv0")
                rebuild_shifts(sr)

            if layers == "conv0":
                for nm, p, n in debug:
                    if nm == "h":
                        nc.sync.dma_start(dbg_d[nm][:], h_buf[:])
                    elif nm == "sr0":
                        nc.sync.dma_start(dbg_d[nm][:], sr[1][:, :n].bitcast(f32))

            # -------------------- residual blocks
            if layers != "conv0":
              with ExitStack() as s1sc:
                wrb = s1sc.enter_context(tc.tile_pool(name="wrb", bufs=1))
                lts = {n: load_w(wrb, n, ci, nt)
                       for n, ci, nt in [("c1a", 128, 9), ("c2a", 64, 1),
                                         ("c1b", 128, 9), ("c2b", 64, 1)]}
                for c1n, c2n in (("c1a", "c2a"), ("c1b", "c2b")):
                    def wb_c1(blk, pt):
                        m0 = 128 * blk
                        nc.scalar.activation(r1_buf[:, m0:m0 + 128], pt[:], AF.Relu)

                    sconv64(lambda blk, t: s_tap(sr, blk, t), lambda blk: s_center(sr, blk),
                            lts[c1n], 512, 9, wb_c1, c1n)

                    def wb_c2(blk, pt):
                        m0 = 128 * blk
                        nc.vector.tensor_tensor(h_buf[:, m0:m0 + 128], h_buf[:, m0:m0 + 128],
                                                pt[:], op=ALU.add)
                        nc.scalar.activation(sr[1][:, 64 + m0:64 + m0 + 128],
                                             h_buf[:, m0:m0 + 128], AF.Relu)

                    sconv64(lambda blk, t: r1_buf[:, 128 * blk:128 * blk + 128],
                            lambda blk: r1_buf[:, 128 * blk:128 * blk + 128],
                            lts[c2n], 1024, 1, wb_c2, c2n)
                    rebuild_shifts(sr)

            if layers == "rb":
                for nm, p, n in debug:
                    if nm == "h":
                        nc.sync.dma_start(dbg_d[nm][:], h_buf[:])
                    elif nm == "sr0":
                        nc.sync.dma_start(dbg_d[nm][:], sr[1][:, :n].bitcast(f32))

        # -------------------- up1 / up2
        if layers in ("up1", "all"):
          with ExitStack() as s2sc:
            x2p = s2sc.enter_context(tc.tile_pool(name="x2p", bufs=1))
            x2_pad = x2p.tile([64, SZ128], f32r, tag="x2_pad")
            nc.sync.dma_start(x2_pad[:], zeros_d[0:64, :SZ128])

            with ExitStack() as su1:
                ltu = load_w(su1.enter_context(tc.tile_pool(name="wu1", bufs=1)), "up1", 128, 9)
                psgp = su1.enter_context(tc.tile_pool(name="psg_u1", bufs=1, space="PSUM"))
                g = gates_64(lambda blk: s_center(sr, blk), ltu, psgp)
                psy = su1.enter_context(tc.tile_pool(name="psy_u1", bufs=1, space="PSUM"))
                pst = su1.enter_context(tc.tile_pool(name="pst_u1", bufs=2, space="PSUM"))
                accp = su1.enter_context(tc.tile_pool(name="acc_u1", bufs=3))
                for blk in range(32):
                    ys = [psy.tile([128, 512], f32, tag=f"yu{ph}", name=f"yu{ph}")
                          for ph in range(4)]
                    for ph in range(4):
                        nc.tensor.matmul(ys[ph][:], ones_s[:],
                                         ltu["b"][:, 512 * ph:512 * (ph + 1)],
                                         start=True, stop=False)
                    for t in range(9):
                        lhsT = s_tap(sr, blk, t)
                        for ph in range(4):
                            nc.tensor.matmul(
                                ys[ph][:], lhsT,
                                ltu["w"][:, (t * 4 + ph) * 512:(t * 4 + ph + 1) * 512],
                                start=False, stop=t == 8)
                    for ph in range(4):
                        di, dj = ph // 2, ph % 2
                        acc = gated_acc(accp, lambda v, ph=ph: ys[ph][:, v * 64:(v + 1) * 64],
                                        g, blk, 64)
                        pt = pst.tile([64, 128], f32, tag="ptu", name="ptu")
                        nc.tensor.transpose(pt[:], acc[:], ident[:])
                        off = 1 + (4 * blk + di + 1) * PW2 + dj
                        nc.scalar.activation(_win2(x2_pad[:], off, 2 * PW2, 2, 2, 64),
                                             pt[:].rearrange("p (r c) -> p r c", r=2),
                                             AF.Relu)

            for nm, p, n in debug:
                if nm == "x2pad":
                    nc.sync.dma_start(dbg_d[nm][:], x2_pad[0:p, :n].bitcast(f32))

            if layers == "all":
              with ExitStack() as su2:
                ltu2 = load_w(su2.enter_context(tc.tile_pool(name="wu2", bufs=1)), "up2", 64, 9)
                g2p = su2.enter_context(tc.tile_pool(name="g2p", bufs=1))
                g2 = g2p.tile([128, 1024], f32, tag="g2", name="g2")
                with ExitStack() as sg2:
                    psgp2 = sg2.enter_context(tc.tile_pool(name="psg_u2", bufs=2, space="PSUM"))
                    for grp in range(4):
                        psg = psgp2.tile([128, 256], f32, tag="psg2", name="psg2")
                        for b2 in range(32):
                            r = 32 * grp + b2
                            lhsT = x2_pad[:, (r + 1) * PW2 + 1:(r + 1) * PW2 + 1 + 128]
                            nc.tensor.matmul(psg[:, 8 * b2:8 * b2 + 8], lhsT, ltu2["cw"][:],
                                             start=True, stop=True)
                        gsl = g2[:, 256 * grp:256 * (grp + 1)]
                        nc.vector.tensor_tensor(gsl, psg[:], ltu2["cb"][:], op=ALU.add)
                        e2 = gpool.tile([128, 256], f32, tag="e2", name="e2")
                        nc.scalar.activation(e2[:], gsl, AF.Exp)
                        ssum = gpool.tile([128, 32], f32, tag="ssum2", name="ssum2")
                        nc.vector.tensor_reduce(ssum[:], e2[:].rearrange("p (g v) -> p g v", v=8),
                                                AX.X, ALU.add)
                        rec = gpool.tile([128, 32], f32, tag="rec2", name="rec2")
                        nc.vector.reciprocal(rec[:], ssum[:])
                        for b2 in range(32):
                            nc.vector.tensor_scalar_mul(gsl[:, 8 * b2:8 * b2 + 8],
                                                        e2[:, 8 * b2:8 * b2 + 8],
                                                        rec[:, b2:b2 + 1])

                psy = su2.enter_context(tc.tile_pool(name="psy_u2", bufs=3, space="PSUM"))
                pst = su2.enter_context(tc.tile_pool(name="pst_u2", bufs=3, space="PSUM"))
                accp = su2.enter_context(tc.tile_pool(name="acc_u2", bufs=3))
                stgp = su2.enter_context(tc.tile_pool(name="stg", bufs=3))
                for r in range(128):
                    y = psy.tile([128, 96], f32, tag="y2", name="y2")
                    nc.tensor.matmul(y[:], ones_s[:], ltu2["b"][:], start=True, stop=False)
                    for t, (dy, dx) in enumerate(TAPS):
                        lhsT = x2_pad[:, (r + dy) * PW2 + dx:(r + dy) * PW2 + dx + 128]
                        nc.tensor.matmul(y[:], lhsT, ltu2["w"][:, t * 96:(t + 1) * 96],
                                         start=False, stop=t == 8)
                    acc = accp.tile([128, 12], f32, tag="acc2", name="acc2")
                    nc.vector.tensor_scalar_mul(acc[:], y[:, 0:12], g2[:, 8 * r:8 * r + 1])
                    for v in range(1, 8):
                        nc.vector.scalar_tensor_tensor(acc[:], y[:, v * 12:(v + 1) * 12],
                                                       g2[:, 8 * r + v:8 * r + v + 1],
                                                       acc[:], ALU.mult, ALU.add)
                    stg = stgp.tile([6, 258], f32, tag="stg", name="stg")
                    for dj in (0, 1):
                        pt = pst.tile([6, 128], f32, tag="pt2", name="pt2")
                        nc.tensor.transpose(pt[:], acc[:, 6 * dj:6 * dj + 6], ident[:])
                        dst = stg[:, dj:dj + 256].rearrange("p (c t) -> p c t", c=128, t=2)[:, :, 0:1]
                        nc.scalar.copy(dst.squeeze(2), pt[:])
                    nc.sync.dma_start(out_d[:, 2 * r:2 * r + 2, :], stg[:, 0:256])

    nc.compile()
    return nc


TRACE = False
_last_results = None


def kernel(**inputs):
    global _last_results
    from concourse import bass_utils
    if "nc" not in _cache:
        _cache["nc"] = _build()
    nc = _cache["nc"]
    wd = _prep_weights(inputs)
    x = np.asarray(inputs["x"], np.float32)
    in_maps = []
    for b in range(B):
        m = {"x": np.ascontiguousarray(x[b].reshape(128, 4096))}
        m.update(wd)
        in_maps.append(m)
    res = bass_utils.run_bass_kernel_spmd(nc, in_maps, core_ids=list(range(8)),
                                          trace=TRACE)
    _last_results = res
    return np.stack([r["out"] for r in res.results])


# revision 24
# speedup vs baseline: 1.0944x; 1.0944x over previous
"""Trainium2 Bass kernel for nn_Decoder_71554155151955 (SwitchedConv decoder).

Data-parallel over batch: 8 batch elements -> 8 NeuronCores, identical program.
Per core: full decoder forward in SBUF, position-major matmuls in float32r.

Design notes:
 - 64-res feature maps are kept channel-major as THREE horizontally shifted
   copies S[dx], dx in {-1, 0, +1}, each [128, 66*64] with zero rows 0 and 65
   (vertical pad) and a zero column folded in by the shift DMAs.  A 3x3 tap
   (dy, dx) for an output block of two rows starting at r0 is then the
   contiguous slice S[dx-1][:, (r0+dy)*64 : +128]  (M=128, single stride).
   S[-1]/S[+1] are rebuilt from S[0] by two SBUF->SBUF DMAs per layer.
 - The up1 output (128-res) is channel-major with a width-129 zero-spacer
   layout: addr(b, c) = 1 + b*129 + c; taps for one 128-wide output row are
   single-stride slices.
 - Main matmuls are position-major: stationary = input positions [K=ci, M=128],
   moving = weights [K=ci, N=(v,co)<=512], taps accumulate in PSUM; a K=1
   matmul against a ones-row seeds the accumulator with the bias.
 - Per-position softmax gates live in [pos, breadth] layout; the breadth-gated
   sum is 8 fused DVE scalar_tensor_tensor ops; PE transposes the gated
   [pos, co] block back to channel-major for the next layer.
 - float32r everywhere on the matmul path (full PE rate, ~11-bit mantissa);
   fp32r PSUM destinations must start at partition 0 (hardware restriction),
   which the M=128 design guarantees.
"""
import numpy as np
from contextlib import ExitStack

B, CIN, H0, W0 = 8, 128, 64, 64
CH, NRC, BR, COUT = 128, 64, 8, 3

SZS = 66 * 64 + 72       # shifted-image buffer: 66 rows x 64 cols + slack
PW2, NB2 = 129, 130      # 128-res spacer layout
SZ128 = 1 + NB2 * PW2 + 260

_cache = {}


def _phase_kernels(W5):
    """[co, ci, 5, 5] -> {(di, dj): [co, ci, 3, 3]} for nearest-up2x + conv."""
    grp = {0: [(0, 0), (1, 0), (2, 1), (3, 1), (4, 2)],
           1: [(0, 0), (1, 1), (2, 1), (3, 2), (4, 2)]}
    out = {}
    for di in (0, 1):
        for dj in (0, 1):
            Wd = np.zeros(W5.shape[:2] + (3, 3), np.float32)
            for ty, s in grp[di]:
                for tx, t in grp[dj]:
                    Wd[:, :, s, t] += W5[:, :, ty, tx]
            out[(di, dj)] = Wd
    return out


def _prep_weights(inputs):
    d = {}

    def conv_layer(name, params, k, fold_bias=False):
        Wt, b, cw, cb = [np.ascontiguousarray(np.asarray(p), np.float32) for p in params]
        br, co, ci, _, _ = Wt.shape
        W = Wt.transpose(3, 4, 2, 0, 1).reshape(k * k, ci, br * co)
        W = np.ascontiguousarray(W.transpose(1, 0, 2).reshape(ci, -1))
        cwT = np.ascontiguousarray(cw[:, :, 0, 0].T)
        if fold_bias:
            # extra all-ones input channel carries conv bias + coupler bias
            W = np.concatenate([W, b.reshape(1, br * co)], 0)
            cwT = np.concatenate([cwT, cb.reshape(1, br)], 0)
            cb = np.zeros_like(cb)
        d[f"w_{name}"] = W
        d[f"b_{name}"] = b.reshape(1, br * co).copy()
        d[f"cw_{name}"] = cwT
        d[f"cb_{name}"] = np.tile(cb.astype(np.float32), (128, 32)).copy()

    conv_layer("conv0", inputs["conv0"], 3)
    conv_layer("c1a", inputs["rb0_c1"], 3)
    conv_layer("c2a", inputs["rb0_c2"], 1, fold_bias=True)
    conv_layer("c1b", inputs["rb1_c1"], 3)
    conv_layer("c2b", inputs["rb1_c2"], 1, fold_bias=True)

    Wt, b, cw, cb = [np.asarray(p).astype(np.float32) for p in inputs["up1"]]
    pk = _phase_kernels(Wt.reshape(BR * 64, CH, 5, 5))
    arr = np.stack([pk[(0, 0)], pk[(0, 1)], pk[(1, 0)], pk[(1, 1)]], 0)
    arr = arr.transpose(3, 4, 2, 0, 1).reshape(9, CH, 4 * 512)  # [(s,t), ci, (ph,v,co)]
    d["w_up1"] = np.ascontiguousarray(arr.transpose(1, 0, 2).reshape(CH, -1))
    d["b_up1"] = np.tile(b.reshape(1, 512), (1, 4)).copy()
    d["cw_up1"] = np.ascontiguousarray(cw[:, :, 0, 0].T)
    d["cb_up1"] = np.tile(cb.astype(np.float32), (128, 32)).copy()

    Wt, b, cw, cb = [np.asarray(p).astype(np.float32) for p in inputs["up2"]]
    pk = _phase_kernels(Wt.reshape(BR * COUT, NRC, 5, 5))
    Bst = np.stack([pk[(0, 0)], pk[(0, 1)], pk[(1, 0)], pk[(1, 1)]], 0)
    Bst = Bst.reshape(2, 2, BR, COUT, NRC, 3, 3)       # (di, dj, v, c, ci, s, t)
    A = Bst.transpose(5, 6, 4, 2, 1, 3, 0)             # (s, t, ci, v, dj, c, di)
    A = np.ascontiguousarray(A).reshape(9, NRC, 96)
    bb = np.zeros((BR, 2, COUT, 2), np.float32)
    bb[:, :, :, :] = b[:, None, :, None]               # (v, dj, c, di)
    brow = np.zeros((9, 1, 96), np.float32)
    brow[4, 0, :] = bb.reshape(96)                     # bias only on center tap
    A = np.concatenate([A, brow], 1)                   # [9, 65, 96]
    d["w_up2"] = np.ascontiguousarray(A.transpose(1, 0, 2).reshape(NRC + 1, -1))
    d["cw_up2"] = np.ascontiguousarray(cw[:, :, 0, 0].T)
    d["cb_up2"] = np.tile(cb.astype(np.float32), (128, 32)).copy()

    d["ones"] = np.ones((1, 128), np.float32)
    d["zeros"] = np.zeros((128, SZ128), np.float32)
    return d


def _win2(ap, off, step_r, nr, step_c, ncnt):
    """2D strided free view [P, nr, ncnt]: addr = off + r*step_r + c*step_c."""
    inner = step_r // step_c
    width = step_r * nr
    v = ap[:, off:off + width].rearrange("p (r c t) -> p r c t",
                                        r=nr, c=inner, t=step_c)
    return v[:, :, 0:ncnt, 0:1].squeeze(3)


def _build(debug=(), layers="all"):
    import concourse.bass as bass
    from concourse import bacc, mybir
    import concourse.tile as tile
    from concourse.masks import make_identity

    f32 = mybir.dt.float32
    f32r = mybir.dt.float32r
    AF = mybir.ActivationFunctionType
    ALU = mybir.AluOpType
    AX = mybir.AxisListType

    nc = bacc.Bacc("TRN2", target_bir_lowering=False, debug=False)

    def din(name, shape, dt=f32r):
        return nc.dram_tensor(name, list(shape), dt, kind="ExternalInput").ap()

    x_d = din("x", (128, 4096))
    NVCO = {"conv0": 1024, "c1a": 512, "c2a": 1024, "c1b": 512, "c2b": 1024,
            "up1": 2048, "up2": 96}
    w = {}
    for name, ci, cwk, ntap, has_b in [
            ("conv0", 128, 128, 9, True), ("c1a", 128, 128, 9, True),
            ("c2a", 65, 65, 1, False), ("c1b", 128, 128, 9, True),
            ("c2b", 65, 65, 1, False), ("up1", 128, 128, 9, True),
            ("up2", 65, 64, 9, False)]:
        w[f"w_{name}"] = din(f"w_{name}", (ci, ntap * NVCO[name]))
        if has_b:
            w[f"b_{name}"] = din(f"b_{name}", (1, NVCO[name]))
        w[f"cw_{name}"] = din(f"cw_{name}", (cwk, 8))
        w[f"cb_{name}"] = din(f"cb_{name}", (128, 256), f32)
    ones_d = din("ones", (1, 128))
    zeros_d = din("zeros", (128, SZ128))
    out_d = nc.dram_tensor("out", [COUT, 256, 256], f32, kind="ExternalOutput").ap()
    dbg_d = {nm: nc.dram_tensor(f"dbg_{nm}", [p, n], f32, kind="ExternalOutput").ap()
             for nm, p, n in debug}

    with tile.TileContext(nc) as tc, ExitStack() as top:
        pers = top.enter_context(tc.tile_pool(name="pers", bufs=1))
        sr = [pers.tile([128, SZS], f32r, tag=f"sr{i}", name=f"sr{i}") for i in range(3)]
        ident = pers.tile([128, 128], f32, tag="ident")
        ones_s = pers.tile([1, 128], f32r, tag="ones")
        gpool = top.enter_context(tc.tile_pool(name="gates", bufs=1))

        zs = pers.tile([128, 144], f32, tag="zs")
        make_identity(nc, ident[:])
        nc.vector.memset(zs[:], 0.0)
        nc.sync.dma_start(ones_s[:], ones_d[:])

        def _col66(ap, off):
            # [128, 66] view of one column (stride 64) at offset `off`
            v = ap[:, off:off + 64 * 66].rearrange("p (r c) -> p r c", r=66, c=64)
            return v[:, :, 0:1].squeeze(2)

        def init_sbuf_zeros(s):
            """zero rows 0 and 65 + tail slack of an S set (engine writes, so
            downstream DMA/matmul wait lists stay short)."""
            sm1, s0, sp1 = s
            for t in (s0, sm1, sp1):
                nc.gpsimd.tensor_copy(t[:, 0:64], zs[:, 0:64])
                nc.gpsimd.tensor_copy(t[:, 65 * 64:SZS], zs[:, 0:64 + 72])

        def rebuild_shifts(s):
            """S[-1], S[+1] from S[0] by whole-buffer shifted copies on the (idle)
            GPSIMD engine; the row-wrap garbage lands exactly on the shift
            zero-columns, which are then re-zeroed by strided writes."""
            sm1, s0, sp1 = s
            nc.gpsimd.tensor_copy(sp1[:, 0:SZS - 1], s0[:, 1:SZS])
            nc.gpsimd.tensor_copy(_col66(sp1[:], 63), zs[:, 0:66])
            nc.gpsimd.tensor_copy(sm1[:, 1:SZS], s0[:, 0:SZS - 1])
            nc.gpsimd.tensor_copy(_col66(sm1[:], 0), zs[:, 0:66])

        def load_w(pool, name, ci, ntap, cwk=None):
            t = {}
            nv = NVCO[name]
            t["w"] = pool.tile([ci, ntap * nv], f32r, tag=f"w_{name}", name=f"w_{name}")
            for tp_ in range(ntap):
                nc.sync.dma_start(t["w"][:, tp_ * nv:(tp_ + 1) * nv],
                                  w[f"w_{name}"][:, tp_ * nv:(tp_ + 1) * nv])
            if f"b_{name}" in w:
                t["b"] = pool.tile([1, nv], f32r, tag=f"b_{name}", name=f"b_{name}")
                nc.sync.dma_start(t["b"][:], w[f"b_{name}"][:])
            t["cw"] = pool.tile([cwk or ci, 8], f32r, tag=f"cw_{name}", name=f"cw_{name}")
            nc.sync.dma_start(t["cw"][:], w[f"cw_{name}"][:])
            t["cb"] = pool.tile([128, 256], f32, tag=f"cb_{name}", name=f"cb_{name}")
            nc.sync.dma_start(t["cb"][:], w[f"cb_{name}"][:])
            return t

        TAPS = [(dy, dx) for dy in range(3) for dx in range(3)]

        def softmax_gates(psg, cb, nblk, gtag="g64"):
            """softmax over groups of 8 of (psg + cb) -> SBUF [128, 8*nblk]."""
            g = gpool.tile([128, 8 * nblk], f32, tag=gtag, name=gtag)
            nc.vector.tensor_tensor(g[:], psg, cb[:, :8 * nblk], op=ALU.add)
            e = gpool.tile([128, 8 * nblk], f32, tag="e_sm", name="e_sm")
            nc.scalar.activation(e[:], g[:], AF.Exp)
            ssum = gpool.tile([128, nblk], f32, tag="ssum", name="ssum")
            nc.vector.tensor_reduce(ssum[:], e[:].rearrange("p (g v) -> p g v", v=8),
                                    AX.X, ALU.add)
            rec = gpool.tile([128, nblk], f32, tag="rec", name="rec")
            nc.vector.reciprocal(rec[:], ssum[:])
            for blk in range(nblk):
                nc.vector.tensor_scalar_mul(g[:, 8 * blk:8 * blk + 8],
                                            e[:, 8 * blk:8 * blk + 8],
                                            rec[:, blk:blk + 1])
            return g

        def gates_group(src_center, lt, psg, g, grp):
            """gates for blocks [8*grp, 8*grp+8) into g[:, 64*grp:...]."""
            for b in range(8 * grp, 8 * grp + 8):
                nc.tensor.matmul(psg[:, 8 * b:8 * b + 8], src_center(b),
                                 lt["cw"][:], start=True, stop=True)
            sl = slice(64 * grp, 64 * grp + 64)
            gsl = g[:, sl]
            nc.vector.tensor_tensor(gsl, psg[:, sl], lt["cb"][:, sl], op=ALU.add)
            e = gpool.tile([128, 64], f32, tag="e_sm", name="e_sm")
            nc.scalar.activation(e[:], gsl, AF.Exp)
            ssum = gpool.tile([128, 8], f32, tag="ssum", name="ssum")
            nc.vector.tensor_reduce(ssum[:], e[:].rearrange("p (g v) -> p g v", v=8),
                                    AX.X, ALU.add)
            rec = gpool.tile([128, 8], f32, tag="rec", name="rec")
            nc.vector.reciprocal(rec[:], ssum[:])
            for i in range(8):
                nc.vector.tensor_scalar_mul(gsl[:, 8 * i:8 * i + 8],
                                            e[:, 8 * i:8 * i + 8], rec[:, i:i + 1])

        def gated_acc(accp, ys, g, blk, co):
            """2-op gating: per-chunk broadcast product into SBUF, then a
            segmented reduce over the breadth axis.  ys: psum tiles whose
            concatenation is [128, (v=8, co)]."""
            nvco = 8 * co
            prod = accp.tile([128, nvco], f32, tag="prod", name="prod")
            pv = prod[:].rearrange("p (c v) -> p v c", v=8)  # (v, c) iter, (c, v) layout
            v0 = 0
            for i, yt in enumerate(ys):
                ncs = yt.shape[1]
                nvc = ncs // co
                gb = g[:, 8 * blk + v0:8 * blk + v0 + nvc].unsqueeze(2) \
                    .to_broadcast([128, nvc, co])
                nc.vector.tensor_tensor(
                    pv[:, v0:v0 + nvc, :],
                    yt[:].rearrange("p (v c) -> p v c", v=nvc), gb, op=ALU.mult)
                v0 += nvc
            acc = accp.tile([128, co], f32, tag="acc", name="acc")
            nc.vector.tensor_reduce(acc[:], prod[:].rearrange("p (c v) -> p c v", v=8),
                                    AX.X, ALU.add)
            return acc

        def sconv64(lhsT_fn, center_fn, lt, nvco, ntap, dst_write, name,
                    seed_bias=True):
            """64-res sconv body.  lhsT_fn(blk, t) -> stationary AP [ci, 128].
            When seed_bias is False the bias is folded into the stationary as
            an extra all-ones input channel (K+1 contraction row)."""
            with ExitStack() as ly:
                psgp = ly.enter_context(tc.tile_pool(name=f"psg_{name}", bufs=1, space="PSUM"))
                psg = psgp.tile([128, 256], f32, tag="psg", name="psg")
                g = gpool.tile([128, 256], f32, tag="g64", name="g64")
                psy = ly.enter_context(tc.tile_pool(name=f"psy_{name}", bufs=2, space="PSUM"))
                pst = ly.enter_context(tc.tile_pool(name=f"pst_{name}", bufs=2, space="PSUM"))
                accp = ly.enter_context(tc.tile_pool(name=f"acc_{name}", bufs=3))
                nchunks = (nvco + 511) // 512
                ncs = nvco // nchunks
                co = nvco // 8
                pending = []

                def flush_one():
                    pblk, pacc = pending.pop(0)
                    pt = pst.tile([co, 128], f32, tag="pt", name="pt")
                    nc.tensor.transpose(pt[:], pacc[:], ident[:])
                    dst_write(pblk, pt)

                for blk in range(32):
                    if blk % 8 == 0:
                        gates_group(center_fn, lt, psg, g, blk // 8)
                    ys = [psy.tile([128, ncs], f32, tag=f"y{c}", name=f"y{c}")
                          for c in range(nchunks)]
                    if seed_bias:
                        for cnk in range(nchunks):
                            nc.tensor.matmul(ys[cnk][:], ones_s[:],
                                             lt["b"][:, ncs * cnk:ncs * (cnk + 1)],
                                             start=True, stop=False)
                    for t in range(ntap):
                        lhsT = lhsT_fn(blk, t)
                        for cnk in range(nchunks):
                            nc.tensor.matmul(
                                ys[cnk][:], lhsT,
                                lt["w"][:, (t * nchunks + cnk) * ncs:(t * nchunks + cnk + 1) * ncs],
                                start=not seed_bias and t == 0, stop=t == ntap - 1)

                    acc = gated_acc(accp, ys, g, blk, co)
                    pending.append((blk, acc))
                    if len(pending) > 1:
                        flush_one()
                for _ in range(len(pending)):
                    flush_one()

        def s_tap(s, blk, t):
            dy, dx = TAPS[t]
            return s[dx][:, (2 * blk + dy) * 64:(2 * blk + dy) * 64 + 128]

        def s_center(s, blk):
            return s[1][:, (2 * blk + 1) * 64:(2 * blk + 1) * 64 + 128]

        init_sbuf_zeros(sr)

        with ExitStack() as hscope:
            hp = hscope.enter_context(tc.tile_pool(name="hp", bufs=1))
            h_buf = hp.tile([128, 4096], f32, tag="h_buf")
            r1_buf = hp.tile([65, 4096], f32r, tag="r1_buf")
            nc.vector.tensor_copy(r1_buf[64:65, :], ones_s[0:1, 0:1].to_broadcast([1, 4096]))

            # -------------------- conv0
            with ExitStack() as s0sc:
                xw = s0sc.enter_context(tc.tile_pool(name="xw", bufs=1))
                sx = [xw.tile([128, SZS], f32r, tag=f"sx{i}", name=f"sx{i}") for i in range(3)]
                init_sbuf_zeros(sx)
                nc.sync.dma_start(sx[1][:, 64:64 + 4096], x_d[:])
                rebuild_shifts(sx)
                lt0 = load_w(s0sc.enter_context(tc.tile_pool(name="w0", bufs=1)), "conv0", 128, 9)

                def wb_conv0(blk, pt):
                    m0 = 128 * blk
                    nc.scalar.copy(h_buf[:, m0:m0 + 128], pt[:])
                    nc.scalar.activation(sr[1][:, 64 + m0:64 + m0 + 128], pt[:], AF.Relu)

                sconv64(lambda blk, t: s_tap(sx, blk, t), lambda blk: s_center(sx, blk),
                        lt0, 1024, 9, wb_conv0, "conv0")
                rebuild_shifts(sr)

            if layers == "conv0":
                for nm, p, n in debug:
                    if nm == "h":
                        nc.sync.dma_start(dbg_d[nm][:], h_buf[:])
                    elif nm == "sr0":
                        nc.sync.dma_start(dbg_d[nm][:], sr[1][:, :n].bitcast(f32))

            # -------------------- residual blocks
            if layers != "conv0":
              with ExitStack() as s1sc:
                wrb = s1sc.enter_context(tc.tile_pool(name="wrb", bufs=1))
                lts = {n: load_w(wrb, n, ci, nt)
                       for n, ci, nt in [("c1a", 128, 9), ("c2a", 65, 1),
                                         ("c1b", 128, 9), ("c2b", 65, 1)]}
                for c1n, c2n in (("c1a", "c2a"), ("c1b", "c2b")):
                    def wb_c1(blk, pt):
                        m0 = 128 * blk
                        nc.scalar.activation(r1_buf[0:64, m0:m0 + 128], pt[:], AF.Relu)

                    sconv64(lambda blk, t: s_tap(sr, blk, t), lambda blk: s_center(sr, blk),
                            lts[c1n], 512, 9, wb_c1, c1n)

                    def wb_c2(blk, pt):
                        m0 = 128 * blk
                        nc.vector.tensor_tensor(h_buf[:, m0:m0 + 128], h_buf[:, m0:m0 + 128],
                                                pt[:], op=ALU.add)
                        nc.scalar.activation(sr[1][:, 64 + m0:64 + m0 + 128],
                                             h_buf[:, m0:m0 + 128], AF.Relu)

                    sconv64(lambda blk, t: r1_buf[:, 128 * blk:128 * blk + 128],
                            lambda blk: r1_buf[:, 128 * blk:128 * blk + 128],
                            lts[c2n], 1024, 1, wb_c2, c2n, seed_bias=False)
                    rebuild_shifts(sr)

            if layers == "rb":
                for nm, p, n in debug:
                    if nm == "h":
                        nc.sync.dma_start(dbg_d[nm][:], h_buf[:])
                    elif nm == "sr0":
                        nc.sync.dma_start(dbg_d[nm][:], sr[1][:, :n].bitcast(f32))

        # -------------------- up1 / up2
        if layers in ("up1", "all"):
          with ExitStack() as s2sc:
            x2p = s2sc.enter_context(tc.tile_pool(name="x2p", bufs=1))
            x2_pad = x2p.tile([65, SZ128], f32r, tag="x2_pad")
            # zero only what up1 writebacks don't cover: top/bottom border rows,
            # the width-129 spacer column, element 0 and the tail slack
            nc.gpsimd.tensor_copy(x2_pad[0:64, 0:1 + PW2],
                                  zs[0:64, 0:1].to_broadcast([64, 1 + PW2]))
            nc.gpsimd.tensor_copy(x2_pad[0:64, 1 + 129 * PW2:SZ128],
                                  zs[0:64, 0:1].to_broadcast([64, SZ128 - 1 - 129 * PW2]))
            spacer = x2_pad[0:64, 1 + PW2 - 1:1 + PW2 - 1 + 129 * PW2] \
                .rearrange("p (r c) -> p r c", r=129, c=PW2)[:, :, 0:1].squeeze(2)
            nc.gpsimd.tensor_copy(spacer, zs[0:64, 0:1].to_broadcast([64, 129]))
            nc.vector.tensor_copy(x2_pad[64:65, :], ones_s[0:1, 0:1].to_broadcast([1, SZ128]))

            with ExitStack() as su1:
                ltu = load_w(su1.enter_context(tc.tile_pool(name="wu1", bufs=1)), "up1", 128, 9)
                psgp = su1.enter_context(tc.tile_pool(name="psg_u1", bufs=1, space="PSUM"))
                psg_u1 = psgp.tile([128, 256], f32, tag="psg", name="psg")
                g = gpool.tile([128, 256], f32, tag="g64", name="g64")
                psy = su1.enter_context(tc.tile_pool(name="psy_u1", bufs=1, space="PSUM"))
                pst = su1.enter_context(tc.tile_pool(name="pst_u1", bufs=2, space="PSUM"))
                accp = su1.enter_context(tc.tile_pool(name="acc_u1", bufs=2))
                pend_u1 = []

                def flush_u1():
                    pblk, ph, pacc = pend_u1.pop(0)
                    di, dj = ph // 2, ph % 2
                    pt = pst.tile([64, 128], f32, tag="ptu", name="ptu")
                    nc.tensor.transpose(pt[:], pacc[:], ident[:])
                    off = 1 + (4 * pblk + di + 1) * PW2 + dj
                    nc.scalar.activation(_win2(x2_pad[0:64, :], off, 2 * PW2, 2, 2, 64),
                                         pt[:].rearrange("p (r c) -> p r c", r=2),
                                         AF.Relu)

                for blk in range(32):
                    if blk % 8 == 0:
                        gates_group(lambda b: s_center(sr, b), ltu, psg_u1, g, blk // 8)
                    ys = [psy.tile([128, 512], f32, tag=f"yu{ph}", name=f"yu{ph}")
                          for ph in range(4)]
                    for ph in range(4):
                        nc.tensor.matmul(ys[ph][:], ones_s[:],
                                         ltu["b"][:, 512 * ph:512 * (ph + 1)],
                                         start=True, stop=False)
                    for t in range(9):
                        lhsT = s_tap(sr, blk, t)
                        for ph in range(4):
                            nc.tensor.matmul(
                                ys[ph][:], lhsT,
                                ltu["w"][:, (t * 4 + ph) * 512:(t * 4 + ph + 1) * 512],
                                start=False, stop=t == 8)
                    for ph in range(4):
                        acc = gated_acc(accp, [ys[ph]], g, blk, 64)
                        pend_u1.append((blk, ph, acc))
                        if len(pend_u1) > 2:
                            flush_u1()
                for _ in range(len(pend_u1)):
                    flush_u1()

            for nm, p, n in debug:
                if nm == "x2pad":
                    nc.sync.dma_start(dbg_d[nm][:], x2_pad[0:p, :n].bitcast(f32))

            if layers == "all":
              with ExitStack() as su2:
                ltu2 = load_w(su2.enter_context(tc.tile_pool(name="wu2", bufs=1)), "up2", 65, 9, cwk=64)
                g2p = su2.enter_context(tc.tile_pool(name="g2p", bufs=1))
                g2 = g2p.tile([128, 1024], f32, tag="g2", name="g2")
                psgp2 = su2.enter_context(tc.tile_pool(name="psg_u2", bufs=1, space="PSUM"))

                def gates2_group(grp):
                    psg = psgp2.tile([128, 256], f32, tag="psg2", name="psg2")
                    for b2 in range(32):
                        r = 32 * grp + b2
                        lhsT = x2_pad[0:64, (r + 1) * PW2 + 1:(r + 1) * PW2 + 1 + 128]
                        nc.tensor.matmul(psg[:, 8 * b2:8 * b2 + 8], lhsT, ltu2["cw"][:],
                                         start=True, stop=True)
                    gsl = g2[:, 256 * grp:256 * (grp + 1)]
                    nc.vector.tensor_tensor(gsl, psg[:], ltu2["cb"][:], op=ALU.add)
                    e2 = gpool.tile([128, 256], f32, tag="e2", name="e2")
                    nc.scalar.activation(e2[:], gsl, AF.Exp)
                    ssum = gpool.tile([128, 32], f32, tag="ssum2", name="ssum2")
                    nc.vector.tensor_reduce(ssum[:], e2[:].rearrange("p (g v) -> p g v", v=8),
                                            AX.X, ALU.add)
                    rec = gpool.tile([128, 32], f32, tag="rec2", name="rec2")
                    nc.vector.reciprocal(rec[:], ssum[:])
                    for b2 in range(32):
                        nc.vector.tensor_scalar_mul(gsl[:, 8 * b2:8 * b2 + 8],
                                                    e2[:, 8 * b2:8 * b2 + 8],
                                                    rec[:, b2:b2 + 1])

                psy = su2.enter_context(tc.tile_pool(name="psy_u2", bufs=3, space="PSUM"))
                pst = su2.enter_context(tc.tile_pool(name="pst_u2", bufs=2, space="PSUM"))
                ps6 = su2.enter_context(tc.tile_pool(name="ps6_u2", bufs=2, space="PSUM"))
                accp = su2.enter_context(tc.tile_pool(name="acc_u2", bufs=3))
                ysbp = su2.enter_context(tc.tile_pool(name="ysb_u2", bufs=3))
                stgp = su2.enter_context(tc.tile_pool(name="stg", bufs=3))
                for chunk in range(32):
                    if chunk % 8 == 0:
                        gates2_group(chunk // 8)
                    # channel-major: stationary = weights [65, 96], moving = x2
                    # positions [65, 4, 128] (4 output rows), N = 512
                    y2 = psy.tile([96, 512], f32, tag="y2", name="y2")
                    for t, (dy, dx) in enumerate(TAPS):
                        off = (4 * chunk + dy) * PW2 + dx
                        mv = x2_pad[:, off:off + 4 * PW2].rearrange(
                            "p (r c) -> p r c", r=4, c=PW2)[:, :, 0:128]
                        nc.tensor.matmul(y2[:], ltu2["w"][:, t * 96:(t + 1) * 96], mv,
                                         start=t == 0, stop=t == 8)
                    ysb = ysbp.tile([96, 512], f32, tag="ysb", name="ysb")
                    nc.scalar.copy(ysb[:], y2[:])
                    for q in range(4):
                        r = 4 * chunk + q
                        pt = pst.tile([128, 96], f32, tag="pt2", name="pt2")
                        nc.tensor.transpose(pt[:], ysb[:, 128 * q:128 * (q + 1)], ident[0:96, 0:96])
                        acc = gated_acc(accp, [pt], g2, r, 12)
                        stg = stgp.tile([6, 258], f32, tag="stg", name="stg")
                        for dj in (0, 1):
                            p6 = ps6.tile([6, 128], f32, tag="p6", name="p6")
                            nc.tensor.transpose(p6[:], acc[:, 6 * dj:6 * dj + 6], ident[:])
                            dst = stg[:, dj:dj + 256].rearrange("p (c t) -> p c t", c=128, t=2)[:, :, 0:1]
                            nc.scalar.copy(dst.squeeze(2), p6[:])
                        nc.sync.dma_start(out_d[:, 2 * r:2 * r + 2, :], stg[:, 0:256])

    nc.compile()
    return nc


TRACE = False
_last_results = None


def kernel(**inputs):
    global _last_results
    from concourse import bass_utils
    if "nc" not in _cache:
        _cache["nc"] = _build()
    nc = _cache["nc"]
    wd = _prep_weights(inputs)
    x = np.asarray(inputs["x"], np.float32)
    in_maps = []
    for b in range(B):
        m = {"x": np.ascontiguousarray(x[b].reshape(128, 4096))}
        m.update(wd)
        in_maps.append(m)
    res = bass_utils.run_bass_kernel_spmd(nc, in_maps, core_ids=list(range(8)),
                                          trace=TRACE)
    _last_results = res
    return np.stack([r["out"] for r in res.results])
